# revision 46
# baseline (speedup 1.0000x reference)
"""Causal self-attention kernel for Trainium2, sharded over 8 NeuronCores.

Problem: x:(2048,2,768) f32, 12 heads, head_dim 64.
Sharding: batch (2) x head-groups (4 groups of 3 heads) -> 8 cores.
Each core computes q/k/v projections for its (batch, 3 heads), causal
flash-style attention, and a partial c_proj contribution. The host sums the
4 partial outputs per batch (the "all-reduce") and adds bo.

Device-side layout notes:
  - Matmul operands are bf16 (fp32 accumulate in PSUM).
  - Scores are computed TRANSPOSED: scoresT[t, s] so softmax's denominator
    comes from a ones-column appended to V (m=65 matmul) and the exp runs
    along the free axis; no PE transposes of the probability matrix needed.
  - Heads h0/h1 score matmuls are row-tiled (PE strips 0:64 / 64:128) and
    write the two banks of one [128,1024] PSUM tile, so ONE fused ACT exp
    covers both heads (halves the per-instruction ACT overhead).
  - c_proj contracts 192 = 128 (full-array matmul) + 64; the 64-row tails
    of an mc-pair run concurrently via row tiling (wo tail + y(h2) are
    duplicated on partitions 64:128 for the second member of each pair).
  - Inputs arrive via a few large contiguous DMAs ordered so the first
    projection matmul can start ~2us in; a short warm-up matmul chain keeps
    the PE HAM clock-gate from idling at 1.2 GHz during the DMA window.
  - Causal masking: diagonal 128x512 score tiles restrict the live column
    range (lo) and a gpsimd affine_select zeroes the triangular remainder
    (one fused call for the h0/h1 pair).
"""

import os
import sys

sys.path.insert(0, "/opt/trn_rl_repo")

import numpy as np

import concourse.bass as bass  # noqa: F401  (import keeps bass registered)
import concourse.tile as tile
from concourse import bacc, bass_utils, library_config, mybir

F32 = mybir.dt.float32
BF16 = mybir.dt.bfloat16

S = 2048          # sequence length
B = 2             # batch
D = 768           # d_model
H = 12            # total heads
HD = 64           # head dim
NH = 3            # heads per core
DKL = NH * HD     # local q/k/v width = 192
KT = 6            # k-tiles over D (6 x 128)
SB = 512          # s-block width
QB = S // SB      # 4 q-blocks
TT = S // 128     # 16 t-tiles
NMC = D // 128    # 6 c_proj row-tiles
SCALE = 1.0 / np.sqrt(HD)

_PROGRAM_CACHE = {}
LAST_EXEC_NS = None


def _build_program():
    nc = bacc.Bacc("TRN2", target_bir_lowering=False, debug=False, num_devices=8)

    # DRAM inputs -- all fully contiguous transfers.
    wq_d = nc.dram_tensor("wq", [128, KT * 256], BF16, kind="ExternalInput").ap()
    wkv_d = nc.dram_tensor("wkv", [128, KT * (128 + 192)], BF16,
                           kind="ExternalInput").ap()
    xt_d = nc.dram_tensor("xt", [QB, 128, KT * SB], BF16, kind="ExternalInput").ap()
    wo1_d = nc.dram_tensor("wo1", [128, D], BF16, kind="ExternalInput").ap()
    wo2_d = nc.dram_tensor("wo2", [128, D], BF16, kind="ExternalInput").ap()
    ones_d = nc.dram_tensor("ones3", [128, 64], BF16, kind="ExternalInput").ap()
    out_d = nc.dram_tensor("outT", [QB, NMC, 128, SB], BF16,
                           kind="ExternalOutput").ap()

    EXP = mybir.ActivationFunctionType.Exp
    GE = mybir.AluOpType.is_ge
    MUL = mybir.AluOpType.mult

    WKOFF = KT * 128          # wv region offset inside wkv tile

    with tile.TileContext(nc) as tc:
        with (
            tc.tile_pool(name="xp", bufs=1) as xp,
            tc.tile_pool(name="wp", bufs=1) as wp,
            tc.tile_pool(name="qk", bufs=1) as qk,
            tc.tile_pool(name="vp", bufs=1) as vp,
            tc.tile_pool(name="ep", bufs=6) as ep,
            tc.tile_pool(name="ys", bufs=1) as ys,
            tc.tile_pool(name="dn", bufs=4) as dn,
            tc.tile_pool(name="op", bufs=4) as op,
            tc.tile_pool(name="psP", bufs=1, space="PSUM") as psP,
            tc.tile_pool(name="psS", bufs=1, space="PSUM") as psS,
            tc.tile_pool(name="psY", bufs=3, space="PSUM") as psY,
            tc.tile_pool(name="psC", bufs=2, space="PSUM") as psC,
        ):
            nc.gpsimd.load_library(library_config.attn)

            # ---- persistent SBUF tiles ----
            wq = wp.tile([128, KT * 256], BF16, tag="wq")
            wkv = wp.tile([128, KT * 320], BF16, tag="wkv")
            wo1 = wp.tile([128, D], BF16, tag="wo1")
            wo2 = wp.tile([128, D], BF16, tag="wo2")  # tail dup'd on parts 64:128
            # one tile per s-chunk, k-tiles side by side -> one contiguous
            # 768KB DMA per chunk (dispatch cost is per-DMA, ~600ns)
            xcs = [xp.tile([128, KT * SB], BF16, tag=f"xc{c}", name=f"xc{c}")
                   for c in range(QB)]

            def xk(k, c):
                return xcs[c][:, k * SB : (k + 1) * SB]

            qA = qk.tile([128, S], BF16, tag="qA")
            qB_ = qk.tile([64, S], BF16, tag="qB")
            kA = qk.tile([128, S], BF16, tag="kA")
            kB = qk.tile([64, S], BF16, tag="kB")
            yA = ys.tile([128, S], BF16, tag="yA")
            yB = ys.tile([128, S], BF16, tag="yB")  # h2 y dup'd on parts 64:128

            vas = []
            for t in range(TT):
                va = vp.tile([128, NH * (HD + 1)], BF16, tag=f"v{t}", name=f"va{t}")
                vas.append(va)
            zz = wp.tile([128, 128], BF16, tag="zz")
            ones3 = wp.tile([128, 64], BF16, tag="ones3")
            onesf = wp.tile([1, 64], F32, tag="onesf")
            exw = wp.tile([1, 4], BF16, tag="exw")

            # ---- warmup seed + ACT table preload during the DMA window ----
            nc.vector.memset(zz[:], 0.0)
            nc.scalar.activation(exw[:], zz[0:1, 0:4], EXP, scale=1.0)

            # ---- input DMAs, ordered for earliest first matmul ----
            nc.sync.dma_start(ones3[:], ones_d)
            nc.vector.tensor_copy(onesf[:], ones3[0:1, :])
            for t in range(TT):
                var = vas[t][:].rearrange("p (h c) -> p h c", c=HD + 1)
                nc.vector.tensor_copy(var[:, :, HD : HD + 1], ones3[:, 0:NH])
            # x chunk 0 is split into per-k-tile sub-DMAs spread over the
            # (otherwise idle) scalar and gpsimd DGE rings: dispatch is ~700ns
            # serial per DMA per ring and transfers are ~128KB, so the g_q
            # k-loop can chase the arrivals; weights ride the sync ring in
            # parallel
            for k in range(3):
                nc.scalar.dma_start(
                    xcs[0][:, k * SB : (k + 1) * SB],
                    xt_d[0][:, k * SB : (k + 1) * SB],
                )
                nc.gpsimd.dma_start(
                    xcs[0][:, (k + 3) * SB : (k + 4) * SB],
                    xt_d[0][:, (k + 3) * SB : (k + 4) * SB],
                )
            nc.sync.dma_start(wq[:], wq_d)
            nc.sync.dma_start(wkv[:], wkv_d)
            for h in range(2):
                nc.scalar.dma_start(
                    xcs[1][:, h * 3 * SB : (h + 1) * 3 * SB],
                    xt_d[1][:, h * 3 * SB : (h + 1) * 3 * SB],
                )
            nc.sync.dma_start(wo1[:], wo1_d)
            nc.sync.dma_start(wo2[:], wo2_d)
            for c in (2, 3):
                for h in range(2):
                    nc.sync.dma_start(
                        xcs[c][:, h * 3 * SB : (h + 1) * 3 * SB],
                        xt_d[c][:, h * 3 * SB : (h + 1) * 3 * SB],
                    )

            # ---- PE warm-up chain: gets the HAM clock-gate to 2.4 GHz while
            # the input DMAs land (needs ~3.4us of sustained PE activity) ----
            psw = psC.tile([128, 128], F32, tag="mm", name="warm")
            NWARM = 38
            for i in range(NWARM):
                nc.tensor.matmul(
                    psw[0:32, :], zz[:, 0:32], zz[:],
                    start=(i == 0), stop=(i == NWARM - 1),
                )

            # ---- emission plan ----
            # Attention is ACT(exp)-paced; projections and c_proj are PE-only.
            # Weave "background" PE groups (next block's projections, previous
            # block's c_proj) between attention tiles so the PE instruction
            # stream never stalls waiting for exp results.
            def qkv_groups(ncol):
                def g_q(n=ncol):
                    ps = psC.tile([128, SB], F32, tag="mm", name=f"pq_{n}")
                    for k in range(KT):
                        nc.tensor.matmul(
                            ps[:], wq[:, k * 256 : k * 256 + 128], xk(k, n),
                            start=(k == 0), stop=(k == KT - 1),
                        )
                    nc.vector.tensor_copy(qA[:, n * SB : (n + 1) * SB], ps[:])

                def g_k(n=ncol):
                    ps = psC.tile([128, SB], F32, tag="mm", name=f"pk_{n}")
                    for k in range(KT):
                        nc.tensor.matmul(
                            ps[:], wkv[:, k * 128 : (k + 1) * 128], xk(k, n),
                            start=(k == 0), stop=(k == KT - 1),
                        )
                    nc.vector.tensor_copy(kA[:, n * SB : (n + 1) * SB], ps[:])

                def g_tail(n=ncol):
                    # one full-array matmul: out rows 0:64 = q cols 128:192,
                    # rows 64:128 = k cols 128:192 (wq carries the k-tail
                    # concatenated at cols 192:256 of each k-tile)
                    ps = psC.tile([128, SB], F32, tag="mm", name=f"pt_{n}")
                    for k in range(KT):
                        nc.tensor.matmul(
                            ps[:], wq[:, k * 256 + 128 : (k + 1) * 256], xk(k, n),
                            start=(k == 0), stop=(k == KT - 1),
                        )
                    nc.vector.tensor_copy(qB_[:, n * SB : (n + 1) * SB], ps[0:64, :])
                    nc.vector.tensor_copy(kB[:, n * SB : (n + 1) * SB], ps[64:128, :])

                def mk_v(t):
                    def g_v():
                        ps = psC.tile([128, SB], F32, tag="mm", name=f"pv{t}")
                        for k in range(KT):
                            nc.tensor.matmul(
                                ps[:, 0:DKL],
                                xk(k, t // 4)[:, (t % 4) * 128 : (t % 4 + 1) * 128],
                                wkv[:, WKOFF + k * 192 : WKOFF + (k + 1) * 192],
                                start=(k == 0), stop=(k == KT - 1),
                            )
                        var = vas[t][:].rearrange("p (h c) -> p h c", c=HD + 1)
                        nc.vector.tensor_copy(
                            var[:, :, 0:HD],
                            ps[:, 0:DKL].rearrange("p (h d) -> p h d", d=HD),
                        )
                    return g_v

                return [g_q, g_k, g_tail] + [mk_v(t) for t in range(4 * ncol, 4 * ncol + 4)]

            def cproj_groups(qb, casts_on_act=False):
                c0, c1 = qb * SB, (qb + 1) * SB

                def mk(mc0, mc1):
                    def g():
                        psa = psC.tile([128, SB], F32, tag="mm", name=f"cp_{qb}_{mc0}")
                        psb = psC.tile([128, SB], F32, tag="mm", name=f"cp_{qb}_{mc1}")
                        nc.tensor.matmul(
                            psa[:], wo1[:, mc0 * 128 : (mc0 + 1) * 128],
                            yA[:, c0:c1], start=True, stop=False,
                        )
                        nc.tensor.matmul(
                            psb[:], wo1[:, mc1 * 128 : (mc1 + 1) * 128],
                            yA[:, c0:c1], start=True, stop=False,
                        )
                        # 64-row tails of the pair run concurrently (row strips
                        # 0:64 and 64:128)
                        nc.tensor.matmul(
                            psa[:], wo2[0:64, mc0 * 128 : (mc0 + 1) * 128],
                            yB[0:64, c0:c1], start=False, stop=True,
                        )
                        nc.tensor.matmul(
                            psb[:], wo2[64:128, mc1 * 128 : (mc1 + 1) * 128],
                            yB[64:128, c0:c1], start=False, stop=True,
                        )
                        for mc, ps in ((mc0, psa), (mc1, psb)):
                            st = op.tile([128, SB], BF16, tag="st",
                                         name=f"st_{qb}_{mc}")
                            if casts_on_act and mc % 2 == 0:
                                # final c_proj: ACT is idle after the last exp;
                                # alternate with DVE so the casts overlap
                                nc.scalar.copy(st[:], ps[:])
                            else:
                                nc.vector.tensor_copy(st[:], ps[:])
                            if casts_on_act and mc % 2 == 1:
                                # spread the last out-DMA dispatches over a
                                # second DGE ring (~600ns serial dispatch each)
                                nc.scalar.dma_start(out_d[qb, mc], st[:])
                            else:
                                nc.sync.dma_start(out_d[qb, mc], st[:])
                    return g
                return [mk(2 * i, 2 * i + 1) for i in range(NMC // 2)]

            def scores_exp_pair(qb, t):
                """h0/h1 scores + fused exp for tile t."""
                d = t * 128 - qb * SB
                lo, sw = (d, 128) if d >= 0 else (0, 0)
                c0 = qb * SB
                tc0, tc1 = t * 128, (t + 1) * 128

                # h0/h1 scores: row-tiled pair into the two banks of psP
                pP = psP.tile([128, 2 * SB], F32, tag="pP", name=f"pP_{qb}_{t}")
                pPv = pP[:].rearrange("p (h c) -> p h c", c=SB)
                nc.tensor.matmul(
                    pP[:, lo:SB], kA[0:64, tc0:tc1],
                    qA[0:64, c0 + lo : c0 + SB], start=True, stop=True,
                )
                nc.tensor.matmul(
                    pP[:, SB + lo : 2 * SB], kA[64:128, tc0:tc1],
                    qA[64:128, c0 + lo : c0 + SB], start=True, stop=True,
                )
                exP = ep.tile([128, 2 * SB], BF16, tag="exP", name=f"xP_{qb}_{t}")
                exPv = exP[:].rearrange("p (h c) -> p h c", c=SB)
                nc.scalar.activation(
                    exPv[:, :, lo:SB], pPv[:, :, lo:SB], EXP, scale=float(SCALE)
                )
                if d >= 0:
                    nc.gpsimd.affine_select(
                        out=exPv[:, :, lo : lo + sw],
                        in_=exPv[:, :, lo : lo + sw],
                        compare_op=GE, fill=0.0,
                        base=0, channel_multiplier=-1,
                        pattern=[[0, 2], [1, sw]],
                    )
                return exP

            def scores_exp_solo(qb, t):
                """h2 scores + exp for tile t."""
                d = t * 128 - qb * SB
                lo, sw = (d, 128) if d >= 0 else (0, 0)
                c0 = qb * SB
                tc0, tc1 = t * 128, (t + 1) * 128

                pS = psS.tile([128, SB], F32, tag="pS", name=f"pS_{qb}_{t}")
                nc.tensor.matmul(
                    pS[:, lo:SB], kB[0:64, tc0:tc1],
                    qB_[0:64, c0 + lo : c0 + SB], start=True, stop=True,
                )
                exS = ep.tile([128, SB], BF16, tag="exS", name=f"xS_{qb}_{t}")
                nc.scalar.activation(
                    exS[:, lo:SB], pS[:, lo:SB], EXP, scale=float(SCALE)
                )
                if d >= 0:
                    nc.gpsimd.affine_select(
                        out=exS[:, lo : lo + sw],
                        in_=exS[:, lo : lo + sw],
                        compare_op=GE, fill=0.0,
                        base=0, channel_multiplier=-1,
                        pattern=[[1, sw]],
                    )
                return exS

            def scores_exp(qb, t):
                exP = scores_exp_pair(qb, t)
                exS = scores_exp_solo(qb, t)
                d = t * 128 - qb * SB
                lo = d if d >= 0 else 0
                return exP, exS, lo

            def attv(t, ex, yps, first, last):
                exP, exS, lo = ex
                nc.tensor.matmul(
                    yps[0][:, lo:SB], vas[t][:, 0 : HD + 1],
                    exP[:, lo:SB], start=first, stop=last,
                )
                nc.tensor.matmul(
                    yps[1][:, lo:SB], vas[t][:, HD + 1 : 2 * (HD + 1)],
                    exP[:, SB + lo : 2 * SB], start=first, stop=last,
                )
                nc.tensor.matmul(
                    yps[2][:, lo:SB], vas[t][:, 2 * (HD + 1) : 3 * (HD + 1)],
                    exS[:, lo:SB], start=first, stop=last,
                )

            def divides(qb, yps):
                c0, c1 = qb * SB, (qb + 1) * SB
                last = qb == QB - 1
                for h in range(NH):
                    dr = dn.tile([1, SB], F32, tag="dr", name=f"dr{qb}{h}")
                    if last:
                        # ACT is idle after the final exp; keep DVE free for
                        # the reciprocal/multiply chain
                        nc.scalar.copy(dr[:], yps[h][HD : HD + 1, :])
                    else:
                        nc.vector.tensor_copy(dr[:], yps[h][HD : HD + 1, :])
                    rc = dn.tile([1, SB], F32, tag="rc", name=f"rc{qb}{h}")
                    nc.vector.reciprocal_approx_fast(rc[:], dr[:])
                    bc = dn.tile([64, SB], F32, tag="bc", name=f"bc{qb}{h}")
                    if last:
                        # final divide: broadcast via a K=1 matmul -- the PE is
                        # otherwise idle here and gpsimd's partition_broadcast
                        # (~1us each, serialized) would pace the endgame
                        pb = psC.tile([64, SB], F32, tag="mm", name=f"pb{qb}{h}")
                        nc.tensor.matmul(
                            pb[0:64, :], onesf[:], rc[:], start=True, stop=True,
                        )
                        nc.vector.tensor_copy(bc[:], pb[0:64, :])
                    else:
                        nc.gpsimd.partition_broadcast(bc[:], rc[:], channels=64)
                    if h == 0:
                        dst = yA[0:64, c0:c1]
                    elif h == 1:
                        dst = yA[64:128, c0:c1]
                    else:
                        dst = yB[0:64, c0:c1]
                    nc.vector.tensor_tensor(dst, yps[h][0:HD, :], bc[:], MUL)
                # duplicate h2's y on partitions 64:128 for the c_proj tails
                nc.vector.tensor_copy(yB[64:128, c0:c1], yB[0:64, c0:c1])

            from collections import deque

            # q/k projections for block 0 go first so attention (and the ACT
            # exp stream) starts as early as possible; block 0's v-groups ride
            # in the background, ordered to match the tile processing order.
            g0 = qkv_groups(0)
            for g in g0[:2]:
                g()
            # g_tail(0) is emitted between the first tile's pair scores and
            # solo scores (the pair only needs g_q/g_k, so its exp -- and the
            # whole ACT stream -- starts ~2.5us earlier)
            gtail0 = g0[2]
            # v-group order matches qb0's tile processing order [0, 3, 2, 1]
            bg = deque([g0[3], g0[6], g0[5], g0[4]])
            for qb in range(QB):
                if qb + 1 < QB:
                    gn = qkv_groups(qb + 1)
                    # v-groups reversed: block qb+1's v tiles are the diagonal
                    # tiles of the next q-block, consumed highest-t first
                    bg.extend(gn[:3] + gn[3:][::-1])
                if qb == QB - 1:
                    # all c_proj work is deferred to the last (largest) q-block,
                    # whose attention stretch is otherwise PE-starved
                    for p in range(QB - 1):
                        bg.extend(cproj_groups(p))
                nt = 4 * qb + 4
                yps = [
                    psY.tile([HD + 1, SB], F32, tag="ya", name=f"yps_{qb}_{h}")
                    for h in range(NH)
                ]
                # tile 0 first (its full-width att@V opens the PSUM accumulation
                # group), then the diagonal (light) tiles while background work
                # is plentiful, ending each q-block on full-width tiles so the
                # PE never idles long enough to re-throttle
                order = [0] + list(range(nt - 1, 0, -1))
                nbg = len(bg)
                emitted = 0
                prev = None
                for i, t in enumerate(order):
                    if qb == 0 and i == 0:
                        exP = scores_exp_pair(qb, t)
                        gtail0()
                        exS = scores_exp_solo(qb, t)
                        ex = (exP, exS, 0)
                    else:
                        ex = scores_exp(qb, t)
                    # software pipeline: this tile's scores/exp are emitted (and
                    # scheduled) ahead of the previous tile's att@V so the PE
                    # always has the next scores ready while ACT runs exp
                    if prev is not None:
                        attv(prev[0], prev[1], yps, prev[2], False)
                    prev = (t, ex, i == 0)
                    # front-load: the bg queue holds the next q-block's
                    # projections, which are prerequisites for its first
                    # scores -- finish them ~2 tiles before the boundary
                    want = (i + 1) * nbg // nt
                    while emitted < want and bg:
                        bg.popleft()()
                        emitted += 1
                attv(prev[0], prev[1], yps, prev[2], True)
                if qb == QB - 1:
                    # keepalive: the final divide chain would otherwise leave
                    # the PE idle long enough for the HAM clock-gate to
                    # re-throttle; dep-free matmul chains (into the now-free
                    # psS bank) bridge it so the final c_proj runs at 2.4 GHz
                    def keep(n, nm):
                        psk = psS.tile([128, SB], F32, tag="pS", name=nm)
                        for i in range(n):
                            nc.tensor.matmul(
                                psk[0:32, 0:128], zz[:, 0:32], zz[:],
                                start=(i == 0), stop=(i == n - 1),
                            )
                    keep(20, "keepA")
                    divides(qb, yps)
                    keep(28, "keepB")
                else:
                    divides(qb, yps)
            for g in cproj_groups(QB - 1, casts_on_act=True):
                g()

    nc.compile()
    return nc


def kernel(x, Wq, bq, Wk, bk, Wv, bv, Wo, bo):
    global LAST_EXEC_NS
    x = np.asarray(x, dtype=np.float32)
    Wq = np.asarray(Wq, dtype=np.float32)
    Wk = np.asarray(Wk, dtype=np.float32)
    Wv = np.asarray(Wv, dtype=np.float32)
    Wo = np.asarray(Wo, dtype=np.float32)
    bq = np.asarray(bq, dtype=np.float32)
    bk = np.asarray(bk, dtype=np.float32)
    bv = np.asarray(bv, dtype=np.float32)
    bo = np.asarray(bo, dtype=np.float32)

    # The device program folds no biases; handle the (unused in this problem)
    # nonzero case on the host by a reference fallback.
    if np.any(bq) or np.any(bk) or np.any(bv):
        q = (x @ Wq + bq).reshape(S, B, H, HD)
        k = (x @ Wk + bk).reshape(S, B, H, HD)
        v = (x @ Wv + bv).reshape(S, B, H, HD)
        att = np.einsum("sbhd,tbhd->bhst", q, k) * SCALE
        causal = np.triu(np.ones((S, S), dtype=bool), k=1)
        att = np.where(causal[None, None], -np.inf, att)
        att = att - att.max(axis=-1, keepdims=True)
        att = np.exp(att)
        att = att / att.sum(axis=-1, keepdims=True)
        y = np.einsum("bhst,tbhd->sbhd", att, v).reshape(S, B, D)
        return (y @ Wo + bo).astype(np.float32)

    if "prog" not in _PROGRAM_CACHE:
        _PROGRAM_CACHE["prog"] = _build_program()
    nc = _PROGRAM_CACHE["prog"]

    import ml_dtypes

    bf = ml_dtypes.bfloat16
    in_maps = []
    xT = [np.ascontiguousarray(x[:, b, :].T).astype(bf) for b in range(B)]
    for c in range(8):
        b, g = c // 4, c % 4
        sl = slice(g * DKL, (g + 1) * DKL)
        Wq_l, Wk_l, Wv_l, Wo_l = Wq[:, sl], Wk[:, sl], Wv[:, sl], Wo[sl, :]

        # wq tile: per k-tile [wq 0:192 | wk-tail 128:192] -> [128, 256]
        wq_full = np.concatenate([Wq_l, Wk_l[:, 128:]], axis=1)  # [768, 256]
        wq_host = np.ascontiguousarray(
            wq_full.reshape(KT, 128, 256).transpose(1, 0, 2).reshape(128, KT * 256)
        ).astype(bf)
        # wkv tile: [wk main cols 0:128 per k | wv 192 per k]
        wk_host = Wk_l[:, :128].reshape(KT, 128, 128)
        wv_host = Wv_l.reshape(KT, 128, 192)
        wkv_host = np.concatenate(
            [
                wk_host.transpose(1, 0, 2).reshape(128, KT * 128),
                wv_host.transpose(1, 0, 2).reshape(128, KT * 192),
            ],
            axis=1,
        ).astype(bf)
        # x chunks: [QB, 128, KT*512] -- one contiguous DMA per s-chunk
        xt_host = np.ascontiguousarray(
            xT[b].reshape(KT, 128, QB, SB).transpose(2, 1, 0, 3).reshape(
                QB, 128, KT * SB)
        )
        wo2_half = Wo_l[128:DKL, :]  # [64, 768]
        in_maps.append({
            "wq": wq_host,
            "wkv": np.ascontiguousarray(wkv_host),
            "xt": xt_host,
            "wo1": np.ascontiguousarray(Wo_l[0:128, :]).astype(bf),
            "wo2": np.ascontiguousarray(
                np.concatenate([wo2_half, wo2_half], axis=0)
            ).astype(bf),
            "ones3": np.ones((128, 4), dtype=bf),
        })

    trace = bool(int(os.environ.get("KERNEL_TRACE", "0")))
    res = bass_utils.run_bass_kernel_spmd(
        nc, in_maps, core_ids=list(range(8)), trace=trace
    )
    LAST_EXEC_NS = res.exec_time_ns

    out = np.zeros((S, B, D), dtype=np.float32)
    for c in range(8):
        b = c // 4
        arr = res.results[c]["outT"].astype(np.float32)  # [QB, NMC, 128, SB]
        full = arr.transpose(1, 2, 0, 3).reshape(D, S)   # [768, 2048]
        out[:, b, :] += full.T
    out += bo
    return out


# revision 50
# speedup vs baseline: 1.0577x; 1.0577x over previous
"""Causal self-attention kernel for Trainium2, sharded over 8 NeuronCores.

Problem: x:(2048,2,768) f32, 12 heads, head_dim 64.
Sharding: batch (2) x head-groups (4 groups of 3 heads) -> 8 cores.
Each core computes q/k/v projections for its (batch, 3 heads), causal
flash-style attention, and a partial c_proj contribution. The host sums the
4 partial outputs per batch (the "all-reduce") and adds bo.

Device-side layout notes:
  - Matmul operands are bf16 (fp32 accumulate in PSUM).
  - Scores are computed TRANSPOSED: scoresT[t, s] so softmax's denominator
    comes from a ones-column appended to V (m=65 matmul) and the exp runs
    along the free axis; no PE transposes of the probability matrix needed.
  - Heads h0/h1 score matmuls are row-tiled (PE strips 0:64 / 64:128) and
    write the two banks of one [128,1024] PSUM tile, so ONE fused ACT exp
    covers both heads (halves the per-instruction ACT overhead).
  - c_proj contracts 192 = 128 (full-array matmul) + 64; the 64-row tails
    of an mc-pair run concurrently via row tiling (wo tail + y(h2) are
    duplicated on partitions 64:128 for the second member of each pair).
  - Inputs arrive via a few large contiguous DMAs ordered so the first
    projection matmul can start ~2us in; a short warm-up matmul chain keeps
    the PE HAM clock-gate from idling at 1.2 GHz during the DMA window.
  - Causal masking: diagonal 128x512 score tiles restrict the live column
    range (lo) and a gpsimd affine_select zeroes the triangular remainder
    (one fused call for the h0/h1 pair).
"""

import os
import sys

sys.path.insert(0, "/opt/trn_rl_repo")

import numpy as np

import concourse.bass as bass  # noqa: F401  (import keeps bass registered)
import concourse.tile as tile
from concourse import bacc, bass_utils, library_config, mybir

F32 = mybir.dt.float32
BF16 = mybir.dt.bfloat16

S = 2048          # sequence length
B = 2             # batch
D = 768           # d_model
H = 12            # total heads
HD = 64           # head dim
NH = 3            # heads per core
DKL = NH * HD     # local q/k/v width = 192
KT = 6            # k-tiles over D (6 x 128)
SB = 512          # s-block width
QB = S // SB      # 4 q-blocks
TT = S // 128     # 16 t-tiles
NMC = D // 128    # 6 c_proj row-tiles
SCALE = 1.0 / np.sqrt(HD)

_PROGRAM_CACHE = {}
LAST_EXEC_NS = None


def _build_program():
    nc = bacc.Bacc("TRN2", target_bir_lowering=False, debug=False, num_devices=8)

    # DRAM inputs -- all fully contiguous transfers.
    wq_d = nc.dram_tensor("wq", [128, KT * 256], BF16, kind="ExternalInput").ap()
    wkv_d = nc.dram_tensor("wkv", [128, KT * (128 + 192)], BF16,
                           kind="ExternalInput").ap()
    xt_d = nc.dram_tensor("xt", [KT, QB, 128, SB], BF16, kind="ExternalInput").ap()
    wo1_d = nc.dram_tensor("wo1", [128, D], BF16, kind="ExternalInput").ap()
    wo2_d = nc.dram_tensor("wo2", [128, D], BF16, kind="ExternalInput").ap()
    ones_d = nc.dram_tensor("ones3", [128, 64], BF16, kind="ExternalInput").ap()
    out_d = nc.dram_tensor("outT", [QB, NMC, 128, SB], BF16,
                           kind="ExternalOutput").ap()

    EXP = mybir.ActivationFunctionType.Exp
    GE = mybir.AluOpType.is_ge
    MUL = mybir.AluOpType.mult

    WKOFF = KT * 128          # wv region offset inside wkv tile

    with tile.TileContext(nc) as tc:
        with (
            tc.tile_pool(name="xp", bufs=1) as xp,
            tc.tile_pool(name="wp", bufs=1) as wp,
            tc.tile_pool(name="qk", bufs=1) as qk,
            tc.tile_pool(name="vp", bufs=1) as vp,
            tc.tile_pool(name="ep", bufs=6) as ep,
            tc.tile_pool(name="ys", bufs=1) as ys,
            tc.tile_pool(name="dn", bufs=4) as dn,
            tc.tile_pool(name="op", bufs=4) as op,
            tc.tile_pool(name="psP", bufs=1, space="PSUM") as psP,
            tc.tile_pool(name="psS", bufs=1, space="PSUM") as psS,
            tc.tile_pool(name="psY", bufs=3, space="PSUM") as psY,
            tc.tile_pool(name="psC", bufs=2, space="PSUM") as psC,
        ):
            nc.gpsimd.load_library(library_config.attn)

            # ---- persistent SBUF tiles ----
            wq = wp.tile([128, KT * 256], BF16, tag="wq")
            wkv = wp.tile([128, KT * 320], BF16, tag="wkv")
            wo1 = wp.tile([128, D], BF16, tag="wo1")
            wo2 = wp.tile([128, D], BF16, tag="wo2")  # tail dup'd on parts 64:128
            # one tile per (k-tile, s-chunk): consumers depend on exactly one
            # 128KB DMA each, so the projection k-loops chase the arrivals
            xts = [[xp.tile([128, SB], BF16, tag=f"x{k}c{c}", name=f"x{k}c{c}")
                    for c in range(QB)] for k in range(KT)]

            def xk(k, c):
                return xts[k][c][:]

            qA = qk.tile([128, S], BF16, tag="qA")
            qB_ = qk.tile([64, S], BF16, tag="qB")
            kA = qk.tile([128, S], BF16, tag="kA")
            kB = qk.tile([64, S], BF16, tag="kB")
            yA = ys.tile([128, S], BF16, tag="yA")
            yB = ys.tile([128, S], BF16, tag="yB")  # h2 y dup'd on parts 64:128

            vas = []
            for t in range(TT):
                va = vp.tile([128, NH * (HD + 1)], BF16, tag=f"v{t}", name=f"va{t}")
                vas.append(va)
            zz = wp.tile([128, 128], BF16, tag="zz")
            ones3 = wp.tile([128, 64], BF16, tag="ones3")
            onesf = wp.tile([1, 64], F32, tag="onesf")
            exw = wp.tile([1, 4], BF16, tag="exw")

            # ---- warmup seed + ACT table preload during the DMA window ----
            nc.vector.memset(zz[:], 0.0)
            nc.scalar.activation(exw[:], zz[0:1, 0:4], EXP, scale=1.0)

            # ---- input DMAs, ordered for earliest first matmul ----
            nc.sync.dma_start(ones3[:], ones_d)
            nc.vector.tensor_copy(onesf[:], ones3[0:1, :])
            for t in range(TT):
                var = vas[t][:].rearrange("p (h c) -> p h c", c=HD + 1)
                nc.vector.tensor_copy(var[:, :, HD : HD + 1], ones3[:, 0:NH])
            # x chunk 0 dispatches ride the (otherwise idle) scalar DGE ring
            # and chunk 1 the gpsimd ring, so they land in parallel with the
            # weight DMAs on the sync ring -- dispatch is ~600ns per DMA and
            # would otherwise serialize the whole startup
            for k in range(KT):
                nc.scalar.dma_start(xts[k][0][:], xt_d[k, 0])
            nc.sync.dma_start(wq[:], wq_d)
            nc.sync.dma_start(wkv[:], wkv_d)
            for k in range(KT):
                nc.gpsimd.dma_start(xts[k][1][:], xt_d[k, 1])
            nc.sync.dma_start(wo1[:], wo1_d)
            nc.sync.dma_start(wo2[:], wo2_d)
            for c in (2, 3):
                for k in range(KT):
                    nc.sync.dma_start(xts[k][c][:], xt_d[k, c])

            # ---- PE warm-up chain: gets the HAM clock-gate to 2.4 GHz while
            # the input DMAs land (needs ~3.4us of sustained PE activity) ----
            psw = psC.tile([128, 128], F32, tag="mm", name="warm")
            NWARM = 38
            for i in range(NWARM):
                nc.tensor.matmul(
                    psw[0:32, :], zz[:, 0:32], zz[:],
                    start=(i == 0), stop=(i == NWARM - 1),
                )

            # ---- emission plan ----
            # Attention is ACT(exp)-paced; projections and c_proj are PE-only.
            # Weave "background" PE groups (next block's projections, previous
            # block's c_proj) between attention tiles so the PE instruction
            # stream never stalls waiting for exp results.
            def qkv_groups(ncol):
                def g_q(n=ncol):
                    ps = psC.tile([128, SB], F32, tag="mm", name=f"pq_{n}")
                    for k in range(KT):
                        nc.tensor.matmul(
                            ps[:], wq[:, k * 256 : k * 256 + 128], xk(k, n),
                            start=(k == 0), stop=(k == KT - 1),
                        )
                    nc.vector.tensor_copy(qA[:, n * SB : (n + 1) * SB], ps[:])

                def g_k(n=ncol):
                    ps = psC.tile([128, SB], F32, tag="mm", name=f"pk_{n}")
                    for k in range(KT):
                        nc.tensor.matmul(
                            ps[:], wkv[:, k * 128 : (k + 1) * 128], xk(k, n),
                            start=(k == 0), stop=(k == KT - 1),
                        )
                    nc.vector.tensor_copy(kA[:, n * SB : (n + 1) * SB], ps[:])

                def g_tail(n=ncol):
                    # one full-array matmul: out rows 0:64 = q cols 128:192,
                    # rows 64:128 = k cols 128:192 (wq carries the k-tail
                    # concatenated at cols 192:256 of each k-tile)
                    ps = psC.tile([128, SB], F32, tag="mm", name=f"pt_{n}")
                    for k in range(KT):
                        nc.tensor.matmul(
                            ps[:], wq[:, k * 256 + 128 : (k + 1) * 256], xk(k, n),
                            start=(k == 0), stop=(k == KT - 1),
                        )
                    nc.vector.tensor_copy(qB_[:, n * SB : (n + 1) * SB], ps[0:64, :])
                    nc.vector.tensor_copy(kB[:, n * SB : (n + 1) * SB], ps[64:128, :])

                def mk_v(t):
                    def g_v():
                        ps = psC.tile([128, SB], F32, tag="mm", name=f"pv{t}")
                        for k in range(KT):
                            nc.tensor.matmul(
                                ps[:, 0:DKL],
                                xk(k, t // 4)[:, (t % 4) * 128 : (t % 4 + 1) * 128],
                                wkv[:, WKOFF + k * 192 : WKOFF + (k + 1) * 192],
                                start=(k == 0), stop=(k == KT - 1),
                            )
                        var = vas[t][:].rearrange("p (h c) -> p h c", c=HD + 1)
                        nc.vector.tensor_copy(
                            var[:, :, 0:HD],
                            ps[:, 0:DKL].rearrange("p (h d) -> p h d", d=HD),
                        )
                    return g_v

                return [g_q, g_k, g_tail] + [mk_v(t) for t in range(4 * ncol, 4 * ncol + 4)]

            def cproj_groups(qb, casts_on_act=False):
                c0, c1 = qb * SB, (qb + 1) * SB

                def mk(mc0, mc1):
                    def g():
                        psa = psC.tile([128, SB], F32, tag="mm", name=f"cp_{qb}_{mc0}")
                        psb = psC.tile([128, SB], F32, tag="mm", name=f"cp_{qb}_{mc1}")
                        nc.tensor.matmul(
                            psa[:], wo1[:, mc0 * 128 : (mc0 + 1) * 128],
                            yA[:, c0:c1], start=True, stop=False,
                        )
                        nc.tensor.matmul(
                            psb[:], wo1[:, mc1 * 128 : (mc1 + 1) * 128],
                            yA[:, c0:c1], start=True, stop=False,
                        )
                        # 64-row tails of the pair run concurrently (row strips
                        # 0:64 and 64:128)
                        nc.tensor.matmul(
                            psa[:], wo2[0:64, mc0 * 128 : (mc0 + 1) * 128],
                            yB[0:64, c0:c1], start=False, stop=True,
                        )
                        nc.tensor.matmul(
                            psb[:], wo2[64:128, mc1 * 128 : (mc1 + 1) * 128],
                            yB[64:128, c0:c1], start=False, stop=True,
                        )
                        for mc, ps in ((mc0, psa), (mc1, psb)):
                            st = op.tile([128, SB], BF16, tag="st",
                                         name=f"st_{qb}_{mc}")
                            if casts_on_act:
                                # final c_proj: ACT is idle after the last exp,
                                # DVE is busy with the divide chain
                                nc.scalar.copy(st[:], ps[:])
                            else:
                                nc.vector.tensor_copy(st[:], ps[:])
                            nc.sync.dma_start(out_d[qb, mc], st[:])
                    return g
                return [mk(2 * i, 2 * i + 1) for i in range(NMC // 2)]

            def scores_exp_pair(qb, t):
                """h0/h1 scores + fused exp for tile t."""
                d = t * 128 - qb * SB
                lo, sw = (d, 128) if d >= 0 else (0, 0)
                c0 = qb * SB
                tc0, tc1 = t * 128, (t + 1) * 128

                # h0/h1 scores: row-tiled pair into the two banks of psP
                pP = psP.tile([128, 2 * SB], F32, tag="pP", name=f"pP_{qb}_{t}")
                pPv = pP[:].rearrange("p (h c) -> p h c", c=SB)
                nc.tensor.matmul(
                    pP[:, lo:SB], kA[0:64, tc0:tc1],
                    qA[0:64, c0 + lo : c0 + SB], start=True, stop=True,
                )
                nc.tensor.matmul(
                    pP[:, SB + lo : 2 * SB], kA[64:128, tc0:tc1],
                    qA[64:128, c0 + lo : c0 + SB], start=True, stop=True,
                )
                exP = ep.tile([128, 2 * SB], BF16, tag="exP", name=f"xP_{qb}_{t}")
                exPv = exP[:].rearrange("p (h c) -> p h c", c=SB)
                nc.scalar.activation(
                    exPv[:, :, lo:SB], pPv[:, :, lo:SB], EXP, scale=float(SCALE)
                )
                if d >= 0:
                    nc.gpsimd.affine_select(
                        out=exPv[:, :, lo : lo + sw],
                        in_=exPv[:, :, lo : lo + sw],
                        compare_op=GE, fill=0.0,
                        base=0, channel_multiplier=-1,
                        pattern=[[0, 2], [1, sw]],
                    )
                return exP

            def scores_exp_solo(qb, t):
                """h2 scores + exp for tile t."""
                d = t * 128 - qb * SB
                lo, sw = (d, 128) if d >= 0 else (0, 0)
                c0 = qb * SB
                tc0, tc1 = t * 128, (t + 1) * 128

                pS = psS.tile([128, SB], F32, tag="pS", name=f"pS_{qb}_{t}")
                nc.tensor.matmul(
                    pS[:, lo:SB], kB[0:64, tc0:tc1],
                    qB_[0:64, c0 + lo : c0 + SB], start=True, stop=True,
                )
                exS = ep.tile([128, SB], BF16, tag="exS", name=f"xS_{qb}_{t}")
                nc.scalar.activation(
                    exS[:, lo:SB], pS[:, lo:SB], EXP, scale=float(SCALE)
                )
                if d >= 0:
                    nc.gpsimd.affine_select(
                        out=exS[:, lo : lo + sw],
                        in_=exS[:, lo : lo + sw],
                        compare_op=GE, fill=0.0,
                        base=0, channel_multiplier=-1,
                        pattern=[[1, sw]],
                    )
                return exS

            def scores_exp(qb, t):
                exP = scores_exp_pair(qb, t)
                exS = scores_exp_solo(qb, t)
                d = t * 128 - qb * SB
                lo = d if d >= 0 else 0
                return exP, exS, lo

            def attv(t, ex, yps, first, last):
                exP, exS, lo = ex
                nc.tensor.matmul(
                    yps[0][:, lo:SB], vas[t][:, 0 : HD + 1],
                    exP[:, lo:SB], start=first, stop=last,
                )
                nc.tensor.matmul(
                    yps[1][:, lo:SB], vas[t][:, HD + 1 : 2 * (HD + 1)],
                    exP[:, SB + lo : 2 * SB], start=first, stop=last,
                )
                nc.tensor.matmul(
                    yps[2][:, lo:SB], vas[t][:, 2 * (HD + 1) : 3 * (HD + 1)],
                    exS[:, lo:SB], start=first, stop=last,
                )

            def divides(qb, yps):
                c0, c1 = qb * SB, (qb + 1) * SB
                last = qb == QB - 1
                for h in range(NH):
                    dr = dn.tile([1, SB], F32, tag="dr", name=f"dr{qb}{h}")
                    if last:
                        # ACT is idle after the final exp; keep DVE free for
                        # the reciprocal/multiply chain
                        nc.scalar.copy(dr[:], yps[h][HD : HD + 1, :])
                    else:
                        nc.vector.tensor_copy(dr[:], yps[h][HD : HD + 1, :])
                    rc = dn.tile([1, SB], F32, tag="rc", name=f"rc{qb}{h}")
                    nc.vector.reciprocal_approx_fast(rc[:], dr[:])
                    bc = dn.tile([64, SB], F32, tag="bc", name=f"bc{qb}{h}")
                    if last:
                        # final divide: broadcast via a K=1 matmul -- the PE is
                        # otherwise idle here and gpsimd's partition_broadcast
                        # (~1us each, serialized) would pace the endgame
                        pb = psC.tile([64, SB], F32, tag="mm", name=f"pb{qb}{h}")
                        nc.tensor.matmul(
                            pb[0:64, :], onesf[:], rc[:], start=True, stop=True,
                        )
                        nc.vector.tensor_copy(bc[:], pb[0:64, :])
                    else:
                        nc.gpsimd.partition_broadcast(bc[:], rc[:], channels=64)
                    if h == 0:
                        dst = yA[0:64, c0:c1]
                    elif h == 1:
                        dst = yA[64:128, c0:c1]
                    else:
                        dst = yB[0:64, c0:c1]
                    nc.vector.tensor_tensor(dst, yps[h][0:HD, :], bc[:], MUL)
                # duplicate h2's y on partitions 64:128 for the c_proj tails
                nc.vector.tensor_copy(yB[64:128, c0:c1], yB[0:64, c0:c1])

            from collections import deque

            # q/k projections for block 0 go first so attention (and the ACT
            # exp stream) starts as early as possible; block 0's v-groups ride
            # in the background, ordered to match the tile processing order.
            g0 = qkv_groups(0)
            for g in g0[:2]:
                g()
            # g_tail(0) is emitted between the first tile's pair scores and
            # solo scores (the pair only needs g_q/g_k, so its exp -- and the
            # whole ACT stream -- starts ~2.5us earlier)
            gtail0 = g0[2]
            # v-group order matches qb0's tile processing order [0, 3, 2, 1]
            bg = deque([g0[3], g0[6], g0[5], g0[4]])
            for qb in range(QB):
                if qb + 1 < QB:
                    gn = qkv_groups(qb + 1)
                    # v-groups reversed: block qb+1's v tiles are the diagonal
                    # tiles of the next q-block, consumed highest-t first
                    bg.extend(gn[:3] + gn[3:][::-1])
                if qb == QB - 1:
                    # all c_proj work is deferred to the last (largest) q-block,
                    # whose attention stretch is otherwise PE-starved
                    for p in range(QB - 1):
                        bg.extend(cproj_groups(p))
                nt = 4 * qb + 4
                yps = [
                    psY.tile([HD + 1, SB], F32, tag="ya", name=f"yps_{qb}_{h}")
                    for h in range(NH)
                ]
                # tile 0 first (its full-width att@V opens the PSUM accumulation
                # group), then the diagonal (light) tiles while background work
                # is plentiful, ending each q-block on full-width tiles so the
                # PE never idles long enough to re-throttle
                order = [0] + list(range(nt - 1, 0, -1))
                nbg = len(bg)
                emitted = 0
                prev = None
                for i, t in enumerate(order):
                    if qb == 0 and i == 0:
                        exP = scores_exp_pair(qb, t)
                        gtail0()
                        exS = scores_exp_solo(qb, t)
                        ex = (exP, exS, 0)
                    else:
                        ex = scores_exp(qb, t)
                    # software pipeline: this tile's scores/exp are emitted (and
                    # scheduled) ahead of the previous tile's att@V so the PE
                    # always has the next scores ready while ACT runs exp
                    if prev is not None:
                        attv(prev[0], prev[1], yps, prev[2], False)
                    prev = (t, ex, i == 0)
                    # front-load: the bg queue holds the next q-block's
                    # projections, which are prerequisites for its first
                    # scores -- finish them ~2 tiles before the boundary
                    want = (i + 1) * nbg // nt
                    while emitted < want and bg:
                        bg.popleft()()
                        emitted += 1
                attv(prev[0], prev[1], yps, prev[2], True)
                if qb == QB - 1:
                    # keepalive: the final divide chain would otherwise leave
                    # the PE idle long enough for the HAM clock-gate to
                    # re-throttle; dep-free matmul chains (into the now-free
                    # psS bank) bridge it so the final c_proj runs at 2.4 GHz
                    def keep(n, nm):
                        psk = psS.tile([128, SB], F32, tag="pS", name=nm)
                        for i in range(n):
                            nc.tensor.matmul(
                                psk[0:32, 0:128], zz[:, 0:32], zz[:],
                                start=(i == 0), stop=(i == n - 1),
                            )
                    keep(20, "keepA")
                    divides(qb, yps)
                    keep(40, "keepB")
                else:
                    divides(qb, yps)
            for g in cproj_groups(QB - 1, casts_on_act=True):
                g()

    nc.compile()
    return nc


def kernel(x, Wq, bq, Wk, bk, Wv, bv, Wo, bo):
    global LAST_EXEC_NS
    x = np.asarray(x, dtype=np.float32)
    Wq = np.asarray(Wq, dtype=np.float32)
    Wk = np.asarray(Wk, dtype=np.float32)
    Wv = np.asarray(Wv, dtype=np.float32)
    Wo = np.asarray(Wo, dtype=np.float32)
    bq = np.asarray(bq, dtype=np.float32)
    bk = np.asarray(bk, dtype=np.float32)
    bv = np.asarray(bv, dtype=np.float32)
    bo = np.asarray(bo, dtype=np.float32)

    # The device program folds no biases; handle the (unused in this problem)
    # nonzero case on the host by a reference fallback.
    if np.any(bq) or np.any(bk) or np.any(bv):
        q = (x @ Wq + bq).reshape(S, B, H, HD)
        k = (x @ Wk + bk).reshape(S, B, H, HD)
        v = (x @ Wv + bv).reshape(S, B, H, HD)
        att = np.einsum("sbhd,tbhd->bhst", q, k) * SCALE
        causal = np.triu(np.ones((S, S), dtype=bool), k=1)
        att = np.where(causal[None, None], -np.inf, att)
        att = att - att.max(axis=-1, keepdims=True)
        att = np.exp(att)
        att = att / att.sum(axis=-1, keepdims=True)
        y = np.einsum("bhst,tbhd->sbhd", att, v).reshape(S, B, D)
        return (y @ Wo + bo).astype(np.float32)

    if "prog" not in _PROGRAM_CACHE:
        _PROGRAM_CACHE["prog"] = _build_program()
    nc = _PROGRAM_CACHE["prog"]

    import ml_dtypes

    bf = ml_dtypes.bfloat16
    in_maps = []
    xT = [np.ascontiguousarray(x[:, b, :].T).astype(bf) for b in range(B)]
    for c in range(8):
        b, g = c // 4, c % 4
        sl = slice(g * DKL, (g + 1) * DKL)
        Wq_l, Wk_l, Wv_l, Wo_l = Wq[:, sl], Wk[:, sl], Wv[:, sl], Wo[sl, :]

        # wq tile: per k-tile [wq 0:192 | wk-tail 128:192] -> [128, 256]
        wq_full = np.concatenate([Wq_l, Wk_l[:, 128:]], axis=1)  # [768, 256]
        wq_host = np.ascontiguousarray(
            wq_full.reshape(KT, 128, 256).transpose(1, 0, 2).reshape(128, KT * 256)
        ).astype(bf)
        # wkv tile: [wk main cols 0:128 per k | wv 192 per k]
        wk_host = Wk_l[:, :128].reshape(KT, 128, 128)
        wv_host = Wv_l.reshape(KT, 128, 192)
        wkv_host = np.concatenate(
            [
                wk_host.transpose(1, 0, 2).reshape(128, KT * 128),
                wv_host.transpose(1, 0, 2).reshape(128, KT * 192),
            ],
            axis=1,
        ).astype(bf)
        # x chunks: [KT, QB, 128, 512], each (k, c) block contiguous
        xt_host = np.ascontiguousarray(
            xT[b].reshape(KT, 128, QB, SB).transpose(0, 2, 1, 3)
        )
        wo2_half = Wo_l[128:DKL, :]  # [64, 768]
        in_maps.append({
            "wq": wq_host,
            "wkv": np.ascontiguousarray(wkv_host),
            "xt": xt_host,
            "wo1": np.ascontiguousarray(Wo_l[0:128, :]).astype(bf),
            "wo2": np.ascontiguousarray(
                np.concatenate([wo2_half, wo2_half], axis=0)
            ).astype(bf),
            "ones3": np.ones((128, 4), dtype=bf),
        })

    trace = bool(int(os.environ.get("KERNEL_TRACE", "0")))
    res = bass_utils.run_bass_kernel_spmd(
        nc, in_maps, core_ids=list(range(8)), trace=trace
    )
    LAST_EXEC_NS = res.exec_time_ns

    out = np.zeros((S, B, D), dtype=np.float32)
    for c in range(8):
        b = c // 4
        arr = res.results[c]["outT"].astype(np.float32)  # [QB, NMC, 128, SB]
        full = arr.transpose(1, 2, 0, 3).reshape(D, S)   # [768, 2048]
        out[:, b, :] += full.T
    out += bo
    return out


# revision 52
# speedup vs baseline: 1.0578x; 1.0001x over previous
"""Causal self-attention kernel for Trainium2, sharded over 8 NeuronCores.

Problem: x:(2048,2,768) f32, 12 heads, head_dim 64.
Sharding: batch (2) x head-groups (4 groups of 3 heads) -> 8 cores.
Each core computes q/k/v projections for its (batch, 3 heads), causal
flash-style attention, and a partial c_proj contribution. The host sums the
4 partial outputs per batch (the "all-reduce") and adds bo.

Device-side layout notes:
  - Matmul operands are bf16 (fp32 accumulate in PSUM).
  - Scores are computed TRANSPOSED: scoresT[t, s] so softmax's denominator
    comes from a ones-column appended to V (m=65 matmul) and the exp runs
    along the free axis; no PE transposes of the probability matrix needed.
  - Heads h0/h1 score matmuls are row-tiled (PE strips 0:64 / 64:128) and
    write the two banks of one [128,1024] PSUM tile, so ONE fused ACT exp
    covers both heads (halves the per-instruction ACT overhead).
  - c_proj contracts 192 = 128 (full-array matmul) + 64; the 64-row tails
    of an mc-pair run concurrently via row tiling (wo tail + y(h2) are
    duplicated on partitions 64:128 for the second member of each pair).
  - Inputs arrive via a few large contiguous DMAs ordered so the first
    projection matmul can start ~2us in; a short warm-up matmul chain keeps
    the PE HAM clock-gate from idling at 1.2 GHz during the DMA window.
  - Causal masking: diagonal 128x512 score tiles restrict the live column
    range (lo) and a gpsimd affine_select zeroes the triangular remainder
    (one fused call for the h0/h1 pair).
"""

import os
import sys

sys.path.insert(0, "/opt/trn_rl_repo")

import numpy as np

import concourse.bass as bass  # noqa: F401  (import keeps bass registered)
import concourse.tile as tile
from concourse import bacc, bass_utils, library_config, mybir

F32 = mybir.dt.float32
BF16 = mybir.dt.bfloat16

S = 2048          # sequence length
B = 2             # batch
D = 768           # d_model
H = 12            # total heads
HD = 64           # head dim
NH = 3            # heads per core
DKL = NH * HD     # local q/k/v width = 192
KT = 6            # k-tiles over D (6 x 128)
SB = 512          # s-block width
QB = S // SB      # 4 q-blocks
TT = S // 128     # 16 t-tiles
NMC = D // 128    # 6 c_proj row-tiles
SCALE = 1.0 / np.sqrt(HD)

_PROGRAM_CACHE = {}
LAST_EXEC_NS = None


def _build_program():
    nc = bacc.Bacc("TRN2", target_bir_lowering=False, debug=False, num_devices=8)

    # DRAM inputs -- all fully contiguous transfers.
    wq_d = nc.dram_tensor("wq", [128, KT * 256], BF16, kind="ExternalInput").ap()
    wkv_d = nc.dram_tensor("wkv", [128, KT * (128 + 192)], BF16,
                           kind="ExternalInput").ap()
    xt_d = nc.dram_tensor("xt", [KT, QB, 128, SB], BF16, kind="ExternalInput").ap()
    wo1_d = nc.dram_tensor("wo1", [128, D], BF16, kind="ExternalInput").ap()
    wo2_d = nc.dram_tensor("wo2", [128, D], BF16, kind="ExternalInput").ap()
    ones_d = nc.dram_tensor("ones3", [128, 64], BF16, kind="ExternalInput").ap()
    out_d = nc.dram_tensor("outT", [QB, NMC, 128, SB], BF16,
                           kind="ExternalOutput").ap()

    EXP = mybir.ActivationFunctionType.Exp
    GE = mybir.AluOpType.is_ge
    MUL = mybir.AluOpType.mult

    WKOFF = KT * 128          # wv region offset inside wkv tile

    with tile.TileContext(nc) as tc:
        with (
            tc.tile_pool(name="xp", bufs=1) as xp,
            tc.tile_pool(name="wp", bufs=1) as wp,
            tc.tile_pool(name="qk", bufs=1) as qk,
            tc.tile_pool(name="vp", bufs=1) as vp,
            tc.tile_pool(name="ep", bufs=6) as ep,
            tc.tile_pool(name="ys", bufs=1) as ys,
            tc.tile_pool(name="dn", bufs=4) as dn,
            tc.tile_pool(name="op", bufs=4) as op,
            tc.tile_pool(name="psP", bufs=1, space="PSUM") as psP,
            tc.tile_pool(name="psS", bufs=1, space="PSUM") as psS,
            tc.tile_pool(name="psY", bufs=3, space="PSUM") as psY,
            tc.tile_pool(name="psC", bufs=2, space="PSUM") as psC,
        ):
            nc.gpsimd.load_library(library_config.attn)

            # ---- persistent SBUF tiles ----
            wq = wp.tile([128, KT * 256], BF16, tag="wq")
            wkv = wp.tile([128, KT * 320], BF16, tag="wkv")
            wo1 = wp.tile([128, D], BF16, tag="wo1")
            wo2 = wp.tile([128, D], BF16, tag="wo2")  # tail dup'd on parts 64:128
            # one tile per (k-tile, s-chunk): consumers depend on exactly one
            # 128KB DMA each, so the projection k-loops chase the arrivals
            xts = [[xp.tile([128, SB], BF16, tag=f"x{k}c{c}", name=f"x{k}c{c}")
                    for c in range(QB)] for k in range(KT)]

            def xk(k, c):
                return xts[k][c][:]

            qA = qk.tile([128, S], BF16, tag="qA")
            qB_ = qk.tile([64, S], BF16, tag="qB")
            kA = qk.tile([128, S], BF16, tag="kA")
            kB = qk.tile([64, S], BF16, tag="kB")
            yA = ys.tile([128, S], BF16, tag="yA")
            yB = ys.tile([128, S], BF16, tag="yB")  # h2 y dup'd on parts 64:128

            vas = []
            for t in range(TT):
                va = vp.tile([128, NH * (HD + 1)], BF16, tag=f"v{t}", name=f"va{t}")
                vas.append(va)
            zz = wp.tile([128, 128], BF16, tag="zz")
            ones3 = wp.tile([128, 64], BF16, tag="ones3")
            onesf = wp.tile([1, 64], F32, tag="onesf")
            exw = wp.tile([1, 4], BF16, tag="exw")

            # ---- warmup seed + ACT table preload during the DMA window ----
            nc.vector.memset(zz[:], 0.0)
            nc.scalar.activation(exw[:], zz[0:1, 0:4], EXP, scale=1.0)

            # ---- input DMAs, ordered for earliest first matmul ----
            nc.sync.dma_start(ones3[:], ones_d)
            nc.vector.tensor_copy(onesf[:], ones3[0:1, :])
            for t in range(TT):
                var = vas[t][:].rearrange("p (h c) -> p h c", c=HD + 1)
                nc.vector.tensor_copy(var[:, :, HD : HD + 1], ones3[:, 0:NH])
            # x chunk 0 is interleaved across the sync and scalar DGE rings
            # (dispatch is ~700ns serial per ring; the gpsimd ring is blocked
            # until ~15us by the async library load) so the g_q k-loop can
            # chase the arrivals from ~8.5us
            nc.scalar.dma_start(xts[0][0][:], xt_d[0, 0])
            nc.sync.dma_start(wq[:], wq_d)
            nc.scalar.dma_start(xts[2][0][:], xt_d[2, 0])
            nc.sync.dma_start(xts[1][0][:], xt_d[1, 0])
            nc.scalar.dma_start(xts[4][0][:], xt_d[4, 0])
            nc.sync.dma_start(xts[3][0][:], xt_d[3, 0])
            nc.sync.dma_start(xts[5][0][:], xt_d[5, 0])
            nc.sync.dma_start(wkv[:], wkv_d)
            for k in range(KT):
                nc.scalar.dma_start(xts[k][1][:], xt_d[k, 1])
            nc.sync.dma_start(wo1[:], wo1_d)
            nc.sync.dma_start(wo2[:], wo2_d)
            for c in (2, 3):
                for k in range(KT):
                    nc.sync.dma_start(xts[k][c][:], xt_d[k, c])

            # ---- PE warm-up chain: gets the HAM clock-gate to 2.4 GHz while
            # the input DMAs land (needs ~3.4us of sustained PE activity) ----
            psw = psC.tile([128, 128], F32, tag="mm", name="warm")
            NWARM = 24
            for i in range(NWARM):
                nc.tensor.matmul(
                    psw[0:32, :], zz[:, 0:32], zz[:],
                    start=(i == 0), stop=(i == NWARM - 1),
                )

            # ---- emission plan ----
            # Attention is ACT(exp)-paced; projections and c_proj are PE-only.
            # Weave "background" PE groups (next block's projections, previous
            # block's c_proj) between attention tiles so the PE instruction
            # stream never stalls waiting for exp results.
            def qkv_groups(ncol):
                def g_q(n=ncol):
                    ps = psC.tile([128, SB], F32, tag="mm", name=f"pq_{n}")
                    for k in range(KT):
                        nc.tensor.matmul(
                            ps[:], wq[:, k * 256 : k * 256 + 128], xk(k, n),
                            start=(k == 0), stop=(k == KT - 1),
                        )
                    nc.vector.tensor_copy(qA[:, n * SB : (n + 1) * SB], ps[:])

                def g_k(n=ncol):
                    ps = psC.tile([128, SB], F32, tag="mm", name=f"pk_{n}")
                    for k in range(KT):
                        nc.tensor.matmul(
                            ps[:], wkv[:, k * 128 : (k + 1) * 128], xk(k, n),
                            start=(k == 0), stop=(k == KT - 1),
                        )
                    nc.vector.tensor_copy(kA[:, n * SB : (n + 1) * SB], ps[:])

                def g_tail(n=ncol):
                    # one full-array matmul: out rows 0:64 = q cols 128:192,
                    # rows 64:128 = k cols 128:192 (wq carries the k-tail
                    # concatenated at cols 192:256 of each k-tile)
                    ps = psC.tile([128, SB], F32, tag="mm", name=f"pt_{n}")
                    for k in range(KT):
                        nc.tensor.matmul(
                            ps[:], wq[:, k * 256 + 128 : (k + 1) * 256], xk(k, n),
                            start=(k == 0), stop=(k == KT - 1),
                        )
                    nc.vector.tensor_copy(qB_[:, n * SB : (n + 1) * SB], ps[0:64, :])
                    nc.vector.tensor_copy(kB[:, n * SB : (n + 1) * SB], ps[64:128, :])

                def mk_v(t):
                    def g_v():
                        ps = psC.tile([128, SB], F32, tag="mm", name=f"pv{t}")
                        for k in range(KT):
                            nc.tensor.matmul(
                                ps[:, 0:DKL],
                                xk(k, t // 4)[:, (t % 4) * 128 : (t % 4 + 1) * 128],
                                wkv[:, WKOFF + k * 192 : WKOFF + (k + 1) * 192],
                                start=(k == 0), stop=(k == KT - 1),
                            )
                        var = vas[t][:].rearrange("p (h c) -> p h c", c=HD + 1)
                        nc.vector.tensor_copy(
                            var[:, :, 0:HD],
                            ps[:, 0:DKL].rearrange("p (h d) -> p h d", d=HD),
                        )
                    return g_v

                return [g_q, g_k, g_tail] + [mk_v(t) for t in range(4 * ncol, 4 * ncol + 4)]

            def cproj_groups(qb, casts_on_act=False):
                c0, c1 = qb * SB, (qb + 1) * SB

                def mk(mc0, mc1):
                    def g():
                        psa = psC.tile([128, SB], F32, tag="mm", name=f"cp_{qb}_{mc0}")
                        psb = psC.tile([128, SB], F32, tag="mm", name=f"cp_{qb}_{mc1}")
                        nc.tensor.matmul(
                            psa[:], wo1[:, mc0 * 128 : (mc0 + 1) * 128],
                            yA[:, c0:c1], start=True, stop=False,
                        )
                        nc.tensor.matmul(
                            psb[:], wo1[:, mc1 * 128 : (mc1 + 1) * 128],
                            yA[:, c0:c1], start=True, stop=False,
                        )
                        # 64-row tails of the pair run concurrently (row strips
                        # 0:64 and 64:128)
                        nc.tensor.matmul(
                            psa[:], wo2[0:64, mc0 * 128 : (mc0 + 1) * 128],
                            yB[0:64, c0:c1], start=False, stop=True,
                        )
                        nc.tensor.matmul(
                            psb[:], wo2[64:128, mc1 * 128 : (mc1 + 1) * 128],
                            yB[64:128, c0:c1], start=False, stop=True,
                        )
                        for mc, ps in ((mc0, psa), (mc1, psb)):
                            st = op.tile([128, SB], BF16, tag="st",
                                         name=f"st_{qb}_{mc}")
                            if casts_on_act:
                                # final c_proj: ACT is idle after the last exp,
                                # DVE is busy with the divide chain
                                nc.scalar.copy(st[:], ps[:])
                            else:
                                nc.vector.tensor_copy(st[:], ps[:])
                            nc.sync.dma_start(out_d[qb, mc], st[:])
                    return g
                return [mk(2 * i, 2 * i + 1) for i in range(NMC // 2)]

            def scores_exp_pair(qb, t):
                """h0/h1 scores + fused exp for tile t."""
                d = t * 128 - qb * SB
                lo, sw = (d, 128) if d >= 0 else (0, 0)
                c0 = qb * SB
                tc0, tc1 = t * 128, (t + 1) * 128

                # h0/h1 scores: row-tiled pair into the two banks of psP
                pP = psP.tile([128, 2 * SB], F32, tag="pP", name=f"pP_{qb}_{t}")
                pPv = pP[:].rearrange("p (h c) -> p h c", c=SB)
                nc.tensor.matmul(
                    pP[:, lo:SB], kA[0:64, tc0:tc1],
                    qA[0:64, c0 + lo : c0 + SB], start=True, stop=True,
                )
                nc.tensor.matmul(
                    pP[:, SB + lo : 2 * SB], kA[64:128, tc0:tc1],
                    qA[64:128, c0 + lo : c0 + SB], start=True, stop=True,
                )
                exP = ep.tile([128, 2 * SB], BF16, tag="exP", name=f"xP_{qb}_{t}")
                exPv = exP[:].rearrange("p (h c) -> p h c", c=SB)
                nc.scalar.activation(
                    exPv[:, :, lo:SB], pPv[:, :, lo:SB], EXP, scale=float(SCALE)
                )
                if d >= 0:
                    nc.gpsimd.affine_select(
                        out=exPv[:, :, lo : lo + sw],
                        in_=exPv[:, :, lo : lo + sw],
                        compare_op=GE, fill=0.0,
                        base=0, channel_multiplier=-1,
                        pattern=[[0, 2], [1, sw]],
                    )
                return exP

            def scores_exp_solo(qb, t):
                """h2 scores + exp for tile t."""
                d = t * 128 - qb * SB
                lo, sw = (d, 128) if d >= 0 else (0, 0)
                c0 = qb * SB
                tc0, tc1 = t * 128, (t + 1) * 128

                pS = psS.tile([128, SB], F32, tag="pS", name=f"pS_{qb}_{t}")
                nc.tensor.matmul(
                    pS[:, lo:SB], kB[0:64, tc0:tc1],
                    qB_[0:64, c0 + lo : c0 + SB], start=True, stop=True,
                )
                exS = ep.tile([128, SB], BF16, tag="exS", name=f"xS_{qb}_{t}")
                nc.scalar.activation(
                    exS[:, lo:SB], pS[:, lo:SB], EXP, scale=float(SCALE)
                )
                if d >= 0:
                    nc.gpsimd.affine_select(
                        out=exS[:, lo : lo + sw],
                        in_=exS[:, lo : lo + sw],
                        compare_op=GE, fill=0.0,
                        base=0, channel_multiplier=-1,
                        pattern=[[1, sw]],
                    )
                return exS

            def scores_exp(qb, t):
                exP = scores_exp_pair(qb, t)
                exS = scores_exp_solo(qb, t)
                d = t * 128 - qb * SB
                lo = d if d >= 0 else 0
                return exP, exS, lo

            def attv(t, ex, yps, first, last):
                exP, exS, lo = ex
                nc.tensor.matmul(
                    yps[0][:, lo:SB], vas[t][:, 0 : HD + 1],
                    exP[:, lo:SB], start=first, stop=last,
                )
                nc.tensor.matmul(
                    yps[1][:, lo:SB], vas[t][:, HD + 1 : 2 * (HD + 1)],
                    exP[:, SB + lo : 2 * SB], start=first, stop=last,
                )
                nc.tensor.matmul(
                    yps[2][:, lo:SB], vas[t][:, 2 * (HD + 1) : 3 * (HD + 1)],
                    exS[:, lo:SB], start=first, stop=last,
                )

            def divides(qb, yps):
                c0, c1 = qb * SB, (qb + 1) * SB
                last = qb == QB - 1
                for h in range(NH):
                    dr = dn.tile([1, SB], F32, tag="dr", name=f"dr{qb}{h}")
                    if last:
                        # ACT is idle after the final exp; keep DVE free for
                        # the reciprocal/multiply chain
                        nc.scalar.copy(dr[:], yps[h][HD : HD + 1, :])
                    else:
                        nc.vector.tensor_copy(dr[:], yps[h][HD : HD + 1, :])
                    rc = dn.tile([1, SB], F32, tag="rc", name=f"rc{qb}{h}")
                    nc.vector.reciprocal_approx_fast(rc[:], dr[:])
                    bc = dn.tile([64, SB], F32, tag="bc", name=f"bc{qb}{h}")
                    if last:
                        # final divide: broadcast via a K=1 matmul -- the PE is
                        # otherwise idle here and gpsimd's partition_broadcast
                        # (~1us each, serialized) would pace the endgame
                        pb = psC.tile([64, SB], F32, tag="mm", name=f"pb{qb}{h}")
                        nc.tensor.matmul(
                            pb[0:64, :], onesf[:], rc[:], start=True, stop=True,
                        )
                        nc.vector.tensor_copy(bc[:], pb[0:64, :])
                    else:
                        nc.gpsimd.partition_broadcast(bc[:], rc[:], channels=64)
                    if h == 0:
                        dst = yA[0:64, c0:c1]
                    elif h == 1:
                        dst = yA[64:128, c0:c1]
                    else:
                        dst = yB[0:64, c0:c1]
                    nc.vector.tensor_tensor(dst, yps[h][0:HD, :], bc[:], MUL)
                # duplicate h2's y on partitions 64:128 for the c_proj tails
                nc.vector.tensor_copy(yB[64:128, c0:c1], yB[0:64, c0:c1])

            from collections import deque

            # q/k projections for block 0 go first so attention (and the ACT
            # exp stream) starts as early as possible; block 0's v-groups ride
            # in the background, ordered to match the tile processing order.
            g0 = qkv_groups(0)
            for g in g0[:2]:
                g()
            # g_tail(0) is emitted between the first tile's pair scores and
            # solo scores (the pair only needs g_q/g_k, so its exp -- and the
            # whole ACT stream -- starts ~2.5us earlier)
            gtail0 = g0[2]
            # v-group order matches qb0's tile processing order [0, 3, 2, 1]
            bg = deque([g0[3], g0[6], g0[5], g0[4]])
            for qb in range(QB):
                if qb + 1 < QB:
                    gn = qkv_groups(qb + 1)
                    # v-groups reversed: block qb+1's v tiles are the diagonal
                    # tiles of the next q-block, consumed highest-t first
                    bg.extend(gn[:3] + gn[3:][::-1])
                if qb == QB - 1:
                    # all c_proj work is deferred to the last (largest) q-block,
                    # whose attention stretch is otherwise PE-starved
                    for p in range(QB - 1):
                        bg.extend(cproj_groups(p))
                nt = 4 * qb + 4
                yps = [
                    psY.tile([HD + 1, SB], F32, tag="ya", name=f"yps_{qb}_{h}")
                    for h in range(NH)
                ]
                # tile 0 first (its full-width att@V opens the PSUM accumulation
                # group), then the diagonal (light) tiles while background work
                # is plentiful, ending each q-block on full-width tiles so the
                # PE never idles long enough to re-throttle
                order = [0] + list(range(nt - 1, 0, -1))
                nbg = len(bg)
                emitted = 0
                prev = None
                for i, t in enumerate(order):
                    if qb == 0 and i == 0:
                        exP = scores_exp_pair(qb, t)
                        gtail0()
                        exS = scores_exp_solo(qb, t)
                        ex = (exP, exS, 0)
                    else:
                        ex = scores_exp(qb, t)
                    # software pipeline: this tile's scores/exp are emitted (and
                    # scheduled) ahead of the previous tile's att@V so the PE
                    # always has the next scores ready while ACT runs exp
                    if prev is not None:
                        attv(prev[0], prev[1], yps, prev[2], False)
                    prev = (t, ex, i == 0)
                    # front-load: the bg queue holds the next q-block's
                    # projections, which are prerequisites for its first
                    # scores -- finish them ~2 tiles before the boundary
                    want = (i + 1) * nbg // nt
                    while emitted < want and bg:
                        bg.popleft()()
                        emitted += 1
                attv(prev[0], prev[1], yps, prev[2], True)
                if qb == QB - 1:
                    # keepalive: the final divide chain would otherwise leave
                    # the PE idle long enough for the HAM clock-gate to
                    # re-throttle; dep-free matmul chains (into the now-free
                    # psS bank) bridge it so the final c_proj runs at 2.4 GHz
                    def keep(n, nm):
                        psk = psS.tile([128, SB], F32, tag="pS", name=nm)
                        for i in range(n):
                            nc.tensor.matmul(
                                psk[0:32, 0:128], zz[:, 0:32], zz[:],
                                start=(i == 0), stop=(i == n - 1),
                            )
                    keep(20, "keepA")
                    divides(qb, yps)
                    keep(40, "keepB")
                else:
                    divides(qb, yps)
            for g in cproj_groups(QB - 1, casts_on_act=True):
                g()

    nc.compile()
    return nc


def kernel(x, Wq, bq, Wk, bk, Wv, bv, Wo, bo):
    global LAST_EXEC_NS
    x = np.asarray(x, dtype=np.float32)
    Wq = np.asarray(Wq, dtype=np.float32)
    Wk = np.asarray(Wk, dtype=np.float32)
    Wv = np.asarray(Wv, dtype=np.float32)
    Wo = np.asarray(Wo, dtype=np.float32)
    bq = np.asarray(bq, dtype=np.float32)
    bk = np.asarray(bk, dtype=np.float32)
    bv = np.asarray(bv, dtype=np.float32)
    bo = np.asarray(bo, dtype=np.float32)

    # The device program folds no biases; handle the (unused in this problem)
    # nonzero case on the host by a reference fallback.
    if np.any(bq) or np.any(bk) or np.any(bv):
        q = (x @ Wq + bq).reshape(S, B, H, HD)
        k = (x @ Wk + bk).reshape(S, B, H, HD)
        v = (x @ Wv + bv).reshape(S, B, H, HD)
        att = np.einsum("sbhd,tbhd->bhst", q, k) * SCALE
        causal = np.triu(np.ones((S, S), dtype=bool), k=1)
        att = np.where(causal[None, None], -np.inf, att)
        att = att - att.max(axis=-1, keepdims=True)
        att = np.exp(att)
        att = att / att.sum(axis=-1, keepdims=True)
        y = np.einsum("bhst,tbhd->sbhd", att, v).reshape(S, B, D)
        return (y @ Wo + bo).astype(np.float32)

    if "prog" not in _PROGRAM_CACHE:
        _PROGRAM_CACHE["prog"] = _build_program()
    nc = _PROGRAM_CACHE["prog"]

    import ml_dtypes

    bf = ml_dtypes.bfloat16
    in_maps = []
    xT = [np.ascontiguousarray(x[:, b, :].T).astype(bf) for b in range(B)]
    for c in range(8):
        b, g = c // 4, c % 4
        sl = slice(g * DKL, (g + 1) * DKL)
        Wq_l, Wk_l, Wv_l, Wo_l = Wq[:, sl], Wk[:, sl], Wv[:, sl], Wo[sl, :]

        # wq tile: per k-tile [wq 0:192 | wk-tail 128:192] -> [128, 256]
        wq_full = np.concatenate([Wq_l, Wk_l[:, 128:]], axis=1)  # [768, 256]
        wq_host = np.ascontiguousarray(
            wq_full.reshape(KT, 128, 256).transpose(1, 0, 2).reshape(128, KT * 256)
        ).astype(bf)
        # wkv tile: [wk main cols 0:128 per k | wv 192 per k]
        wk_host = Wk_l[:, :128].reshape(KT, 128, 128)
        wv_host = Wv_l.reshape(KT, 128, 192)
        wkv_host = np.concatenate(
            [
                wk_host.transpose(1, 0, 2).reshape(128, KT * 128),
                wv_host.transpose(1, 0, 2).reshape(128, KT * 192),
            ],
            axis=1,
        ).astype(bf)
        # x chunks: [KT, QB, 128, 512], each (k, c) block contiguous
        xt_host = np.ascontiguousarray(
            xT[b].reshape(KT, 128, QB, SB).transpose(0, 2, 1, 3)
        )
        wo2_half = Wo_l[128:DKL, :]  # [64, 768]
        in_maps.append({
            "wq": wq_host,
            "wkv": np.ascontiguousarray(wkv_host),
            "xt": xt_host,
            "wo1": np.ascontiguousarray(Wo_l[0:128, :]).astype(bf),
            "wo2": np.ascontiguousarray(
                np.concatenate([wo2_half, wo2_half], axis=0)
            ).astype(bf),
            "ones3": np.ones((128, 4), dtype=bf),
        })

    trace = bool(int(os.environ.get("KERNEL_TRACE", "0")))
    res = bass_utils.run_bass_kernel_spmd(
        nc, in_maps, core_ids=list(range(8)), trace=trace
    )
    LAST_EXEC_NS = res.exec_time_ns

    out = np.zeros((S, B, D), dtype=np.float32)
    for c in range(8):
        b = c // 4
        arr = res.results[c]["outT"].astype(np.float32)  # [QB, NMC, 128, SB]
        full = arr.transpose(1, 2, 0, 3).reshape(D, S)   # [768, 2048]
        out[:, b, :] += full.T
    out += bo
    return out
